# revision 56
# baseline (speedup 1.0000x reference)
"""Trainium2 Bass kernel: parity-polynomial segment_reduce.

Reference math:
    spins = 1 - 2*bits                                   # {-1,+1}
    parities[b,t] = prod_o spins_pad[b, idx_pad[t,o]]    # [B, T]
    out[b] = parities[b] @ theta

Every parity factor is (-1)^{bit}, so
    out[b] = sum_t theta[t] * (-1)^{popcount(key[b] & mask[t])}
with key[b] = sum_i bits[b,i]<<i and mask[t] = XOR-fold of (1<<idx_pad[t,o]).
For this problem every mask < 4096, so with key = (p<<6)|c, mask = (q<<6)|d:

    out[b] = sum_{q,d} TsM[q,d] * H[q,p_b] * H[d,c_b]
           = sum_d v[d,b] * M1[d,b]

where TsM = theta_spread.reshape(64,64), H = Sylvester-Hadamard-64,
A = (-2H) @ TsM (the only theta-side compute: ONE 64x64 matmul),
M1[d,b] = A[p_b,d] (a one-hot gather matmul over the p-side), and
v[d,b] = (-1)^{popcount(d & c_b)} enters as vhat = parity(pc) - 0.5 = -v/2
(the -2 is folded into H on the host).

parity(pc) is built EXACTLY with fp32 round-to-nearest-even (HW-verified;
`mod` is not a legal tensor_scalar op on TRN2):
  - a third key matmul computes q1 = pc/2 - 0.25 + 2^23 whose fp32 PSUM
    accumulation rounds to exactly 2^23 + floor(pc/2) (the two constants ride
    two bf16-exact ones-rows, the big one last in partition order);
  - s1 = -2*q1 + 2^24 = -2*floor(pc/2)            (Activation, Copy w/ scale)
  - vhat = (pc - 0.5) + s1 = parity - 0.5          (DVE scalar_tensor_tensor)

The 512-sample batch is split into two groups whose bit rows sit at matmul
partition bases 0/32; each key matmul writes its group's PSUM partition half
directly ([128,256] stacks; a 64-col stationary may target out-partition base
64, while ACCUMULATING across different stationary bases hangs the HW).  The
stacked layout makes is_equal, vhat, and the product ONE 128-partition DVE op
each (DVE cost scales with the free axis only).

Per core (512 batch rows):
  1. ONE input DMA [64, 576] bf16 (SP/HWDGE): row-stacked bit groups,
     the three 64-col key stationaries, -2*H64, TsM.
  2. PE: warm-ups seed the p-state ramp; A-matmul; 6 key matmuls -> p_k
     (p(b)-j offsets), p_q1 (rounded half-counts), p_vk (popcount counts).
  3. DVE: stages A into the two gather stationary blocks (separate tiles:
     same-tile WAW costs a blocking self-sem), is_equal -> OHp stack,
     vhat via scalar_tensor_tensor; Act: s1 (its only op, so the scheduler
     cannot reorder it behind copies).
  4. PE: two gather matmuls -> M1 stack (bases 0/64); DVE: prod = M1 * vhat
     (a tiny spacer op sits between vhat and prod: Tile adds a blocking
     self-sem when an op reads its IMMEDIATE DVE predecessor's output);
     PE: four 1-column ones-matmuls column-reduce prod into p_out[128, 4]
     (sample g*128+i lands on partition i, column g).
  5. DVE stages p_out -> SBUF; a PREPARED kv_writeback (descriptors generated
     on Pool during the input-DMA wait) is fired by trigger_dma: the
     post-compute tail is trigger + transfer + DMA-sem, skipping the 625ns
     HWDGE gen + 650ns DGE->DMA delay of a plain dma_start.

Host does only sharding, dtype/layout staging, and index bookkeeping
(mask XOR-fold + theta scatter).  All theta- and bit-dependent arithmetic
runs on device.
"""

import numpy as np

B, NUM_BITS, ORDER = 4096, 32, 12
N_CORES = 8
B_LOCAL = B // N_CORES          # 512
KEYS = 1 << ORDER               # 4096
PC = 64                         # 6/6 split: p = key>>6, c = key&63
PC_BITS = 6
GROUPS = 2                      # batch groups row-stacked at partition 0 / 32
GROW = 32                       # group g rows start at g*GROW (matmul base rule)
GB = B_LOCAL // GROUPS          # 256 samples per group
ROWS = PC_BITS * 2 + 2          # 12 bit rows + two ones rows = 14
IN_COLS = GB + 5 * PC           # bits 256 | statP 64 | statQ 64 | statC 64 | H | Ts

_STATE = {}


def _sylvester(n):
    """H[i,j] = (-1)^popcount(i&j), Sylvester ordering."""
    h = np.array([[1.0]], dtype=np.float32)
    while h.shape[0] < n:
        h = np.block([[h, h], [h, -h]])
    return np.ascontiguousarray(h, dtype=np.float32)


def _build_module():
    import bass_rust as _bass_rust
    import concourse.mybir as mybir
    import concourse.tile as tile
    from concourse import bacc

    f32 = mybir.dt.float32
    bf16 = mybir.dt.bfloat16
    i32 = mybir.dt.int32
    nc = bacc.Bacc(
        "TRN2",
        target_bir_lowering=False,
        debug=False,
        enable_asserts=True,
        num_devices=N_CORES,
    )

    inp = nc.dram_tensor("inp", [PC, IN_COLS], bf16, kind="ExternalInput").ap()
    # kv layout [batch=1, d_head=128, d_head_outer=1, n_ctx=4] is flat
    # f32[p*4+g] == p_out[p, g], identical to a plain [128, 4] tensor.
    out = nc.dram_tensor("out", [1, 128, 1, 4], f32, kind="ExternalOutput").ap()

    C_SP = GB                  # stat-P block (p-offsets)
    C_SQ = GB + PC             # stat-Q block: pc/2 - 0.25 + 2^23 (RNE floor)
    C_SC = GB + 2 * PC         # stat-C block (popcount weights)
    C_H = GB + 3 * PC          # -2*H64
    C_TS = GB + 4 * PC         # TsM

    with tile.TileContext(nc) as tc:
        with (
            tc.tile_pool(name="sb", bufs=1) as sb,
            tc.tile_pool(name="ps", bufs=1, space="PSUM") as ps,
        ):
            # --- pre-DMA setup: constants + PE p-state warm-up ---------------
            t_w = sb.tile([1, 1], bf16)
            nc.vector.memset(t_w, 1.0)
            t_ones = sb.tile([128, 1], f32)
            nc.vector.memset(t_ones, 1.0)
            t_A0 = sb.tile([PC, PC], bf16)
            t_A1 = sb.tile([128, PC], bf16)
            t_out = sb.tile([128, 1, 1, 4], f32)
            t_ctx = sb.tile([128, 1], i32)
            nc.gpsimd.memset(t_ctx, 0)
            dma_sem = nc.alloc_semaphore("out_dma")
            nc.gpsimd.kv_writeback(
                out, t_out[:], t_ctx[:], prepare_only=True, sem=dma_sem)
            # warm-ups target p_out (overwritten by the column sums later);
            # PSUM has only 8 banks and each tile takes a full bank
            p_out = ps.tile([128, 4], f32)
            nc.tensor.matmul(p_out[0:1, 0:1], t_w, t_w)
            nc.tensor.matmul(p_out[0:1, 0:1], t_w, t_w)

            t_in = sb.tile([PC, IN_COLS], bf16)
            nc.sync.dma_start(out=t_in, in_=inp)

            # --- theta side: A = (-2H) @ TsM, staged into the two diagonal
            # blocks of the M1 stationary (Act; GPSIMD can't touch PSUM)
            p_A = ps.tile([PC, PC], f32)
            nc.tensor.matmul(p_A, t_in[:, C_H : C_H + PC], t_in[:, C_TS : C_TS + PC])
            # both gather stationary blocks staged by DVE in its idle window
            # before is_equal's input is ready (the second p_A read carries a
            # ~160ns reader-chain guard on ANY engine; DVE's window hides it)
            nc.vector.tensor_copy(t_A0, p_A)
            nc.vector.tensor_copy(t_A1[PC : 2 * PC, :], p_A)

            # --- bits side: key matmuls stack group0 on PSUM partitions
            # 0:64 and group1 on 64:128 (64-col stationaries; the PSUM out AP
            # carries the partition base -- HW-verified, accumulation across
            # different stationary bases is NOT)
            p_k = ps.tile([2 * PC, GB], f32)
            p_q1 = ps.tile([2 * PC, GB], f32)
            p_vk = ps.tile([2 * PC, GB], f32)
            r0 = slice(0, ROWS)
            r1 = slice(GROW, GROW + ROWS)
            for p_dst, c_base in ((p_q1, C_SQ), (p_k, C_SP), (p_vk, C_SC)):
                nc.tensor.matmul(p_dst[0:PC, :], t_in[r0, c_base : c_base + PC],
                                 t_in[r0, 0:GB])
                nc.tensor.matmul(p_dst[PC : 2 * PC, :],
                                 t_in[r1, c_base : c_base + PC],
                                 t_in[r1, 0:GB])

            t_oh = sb.tile([2 * PC, GB], bf16)
            nc.vector.tensor_scalar(
                out=t_oh, in0=p_k,
                scalar1=0.0, scalar2=None, op0=mybir.AluOpType.is_equal)
            # sign contraction operand: vhat = parity(pc) - 0.5, parity from
            # the fp32 round-to-nearest-even floor baked into the q1 matmul:
            #   q1(PSUM) = 2^23 + floor(pc/2)   (exactly; HW-verified)
            #   s1 = q1*-2 + 2^24 = -2*floor(pc/2)
            #   vhat = (pc - 0.5) + s1 = parity - 0.5  (exact, in {-.5, +.5})
            t_s1 = sb.tile([2 * PC, GB], f32)
            nc.scalar.activation(
                t_s1, p_q1, mybir.ActivationFunctionType.Copy,
                bias=float(2.0**24), scale=-2.0)
            t_vst = sb.tile([2 * PC, GB], f32)
            nc.vector.scalar_tensor_tensor(
                out=t_vst, in0=p_vk, scalar=-0.5, in1=t_s1,
                op0=mybir.AluOpType.add, op1=mybir.AluOpType.add)

            # --- gather + sign-contraction: out[b] = sum_d M1[d,b]*v[d,b] ----
            p_M1 = ps.tile([2 * PC, GB], f32)
            nc.tensor.matmul(p_M1[PC : 2 * PC, :], t_A1[PC : 2 * PC, :],
                             t_oh[PC : 2 * PC, :])
            nc.tensor.matmul(p_M1[0:PC, :], t_A0, t_oh[0:PC, :])
            # tiny spacer op: Tile inserts a blocking self-sem when an op
            # reads its IMMEDIATE DVE predecessor's output; with the spacer
            # the vhat write is two ops back and prod dispatches freely
            t_dmy = sb.tile([1, 1], f32)
            nc.vector.tensor_copy(t_dmy, t_s1[0:1, 0:1])
            t_prod = sb.tile([2 * PC, GB], f32)
            nc.vector.tensor_mul(t_prod, p_M1, t_vst)

            # column sums, batch transposed onto output partitions: four
            # 1-column matmuls (stationary = a 128-column slice of prod)
            for g in range(4):
                rows = slice((g // 2) * PC, (g // 2 + 1) * PC)
                cols = slice((g % 2) * 128, (g % 2 + 1) * 128)
                nc.tensor.matmul(p_out[:, g : g + 1],
                                 t_prod[rows, cols], t_ones[rows, :])

            # the trigger is sequencer-only and Tile's deferred-RAW machinery
            # only covers producers issued BEFORE the prep, so attach an
            # explicit sync dependency on the staging copy
            cp = nc.vector.tensor_copy(t_out[:, 0, 0, :], p_out)
            trig = nc.gpsimd.trigger_dma(count=None)
            deps = _bass_rust.InstructionNameOrderedSet()
            deps.add(cp.ins.name)
            trig.ins.add_sync_dependencies_from(deps)
            wt = nc.gpsimd.wait_ge(dma_sem, 16)
            tdeps = _bass_rust.InstructionNameOrderedSet()
            tdeps.add(trig.ins.name)
            wt.ins.add_nosync_dependencies_from(tdeps)

    # The exit block ends with TWO full all-engine barrier rounds; round 2
    # only fences the sem-range-clear against engine halt, but every engine's
    # stream ends right after round 1 (nothing races the clear, launches are
    # serialized by all-engines-halted, and round 1 leaves the barrier sems
    # at 0 -- the state the next launch's entry expects).  Truncate after the
    # clear (the last Pool ISA instruction).
    end_blk = list(nc.m.functions[0].blocks)[-1]
    il = end_blk.instructions
    isa_idx = max(i for i, inst in enumerate(il)
                  if type(inst).__name__.endswith("InstISA"))
    while len(il) > isa_idx + 1:
        il.pop()

    # Tile put the kv_writeback prep on the DMASW0 lane and emitted exit
    # waits on that lane's sem, but with a user completion sem (sem=) nothing
    # ever bumps it -> deadlock.  Pool's explicit wait_ge(out_dma)>=16 plus
    # the final all-engine barrier already guarantee the transfer landed
    # before teardown, so drop the orphaned lane waits.
    for blk in nc.m.functions[0].blocks:
        for inst in blk.instructions:
            si = inst.sync_info
            if si is None or not si.on_wait:
                continue
            if any(w.ant_name and w.ant_name.startswith("DMASW")
                   for w in si.on_wait):
                si.on_wait = [
                    w for w in si.on_wait
                    if not (w.ant_name and w.ant_name.startswith("DMASW"))
                ]

    nc.compile()
    return nc


def _get_module():
    nc = _STATE.get("nc")
    if nc is None:
        nc = _build_module()
        _STATE["nc"] = nc
    return nc


def _host_prep(bitstrings, theta, idx_pad):
    """Index bookkeeping + input staging. Returns per-core input maps."""
    import ml_dtypes

    bitstrings = np.asarray(bitstrings)
    theta = np.asarray(theta, dtype=np.float32)
    idx_pad = np.asarray(idx_pad).astype(np.int64)

    # mask[t] = XOR-fold of one-hot bit positions (pad index >= NUM_BITS -> no bit)
    onehots = np.where(idx_pad >= NUM_BITS, 0, np.int64(1) << np.clip(idx_pad, 0, 62))
    masks = np.bitwise_xor.reduce(onehots, axis=1)
    if masks.size and int(masks.max()) >= KEYS:
        raise NotImplementedError(
            "kernel specialized for masks spanning bits 0..11 "
            f"(max mask {int(masks.max())})"
        )
    theta_spread = np.zeros(KEYS, np.float32)
    np.add.at(theta_spread, masks, theta)
    ts_m = theta_spread.reshape(PC, PC)                 # TsM[q, d]

    # Key-matmul stationaries [14, 64] each (rows 12/13 are ones-rows):
    #   p-off[:, j] : p(b)-j  = sum_{k=6..11} 2^(k-6) bit_k  +  (-j)*1
    #   cnt[:, d]   : pc = popcount(d & c_b) = sum_{k=0..5} dbit_k bit_k
    #   q1[:, d]    : pc/2 - 0.25 + 2^23 -> fp32 PSUM rounds (RNE) to
    #                 2^23 + floor(pc/2); constants split across the two
    #                 ones-rows (each bf16-exact), 2^23 last in partition
    #                 order so the single rounding happens at the end
    w_p = np.zeros((ROWS, PC), np.float32)
    for k in range(PC_BITS):
        w_p[PC_BITS + k, :] = float(1 << k)
    w_p[2 * PC_BITS, :] = -np.arange(PC, dtype=np.float32)
    d_idx = np.arange(PC)
    w_c = np.zeros((ROWS, PC), np.float32)
    for k in range(PC_BITS):
        w_c[k, :] = ((d_idx >> k) & 1).astype(np.float32)
    w_q = 0.5 * w_c
    w_q[2 * PC_BITS, :] = -0.25
    w_q[2 * PC_BITS + 1, :] = float(2.0**23)

    h64 = _sylvester(PC)

    C_SP = GB
    C_SQ = GB + PC
    C_SC = GB + 2 * PC
    C_H = GB + 3 * PC
    C_TS = GB + 4 * PC

    base = np.zeros((PC, IN_COLS), np.float32)
    base[:, C_H : C_H + PC] = -2.0 * h64
    base[:, C_TS : C_TS + PC] = ts_m
    # group g's stationaries live at rows g*GROW (sharing the moving base)
    for g in range(GROUPS):
        rows = slice(g * GROW, g * GROW + ROWS)
        base[rows, C_SP : C_SP + PC] = w_p
        base[rows, C_SQ : C_SQ + PC] = w_q
        base[rows, C_SC : C_SC + PC] = w_c

    bits_f = bitstrings[:, :ORDER].astype(np.float32)
    in_maps = []
    for c in range(N_CORES):
        buf = base.copy()
        for g in range(GROUPS):
            rows = slice(g * GROW, g * GROW + ORDER)
            s0 = c * B_LOCAL + g * GB
            buf[rows, 0:GB] = bits_f[s0 : s0 + GB, :].T
            buf[g * GROW + ORDER, 0:GB] = 1.0
            buf[g * GROW + ORDER + 1, 0:GB] = 1.0
        in_maps.append({"inp": buf.astype(ml_dtypes.bfloat16)})
    return in_maps


def kernel(bitstrings, theta, idx_pad):
    from concourse.bass_utils import run_bass_kernel_spmd

    in_maps = _host_prep(bitstrings, theta, idx_pad)
    nc = _get_module()
    res = run_bass_kernel_spmd(nc, in_maps, core_ids=list(range(N_CORES)))
    # out flat f32[i*4 + g] holds sample b_local = g*128 + i
    out = np.concatenate(
        [np.asarray(r["out"]).reshape(128, 4).T.ravel() for r in res.results])
    return out.astype(np.float32)


# revision 57
# speedup vs baseline: 1.0470x; 1.0470x over previous
"""Trainium2 Bass kernel: parity-polynomial segment_reduce.

Reference math:
    spins = 1 - 2*bits                                   # {-1,+1}
    parities[b,t] = prod_o spins_pad[b, idx_pad[t,o]]    # [B, T]
    out[b] = parities[b] @ theta

Every parity factor is (-1)^{bit}, so
    out[b] = sum_t theta[t] * (-1)^{popcount(key[b] & mask[t])}
with key[b] = sum_i bits[b,i]<<i and mask[t] = XOR-fold of (1<<idx_pad[t,o]).
For this problem every mask < 4096, so with key = (p<<6)|c, mask = (q<<6)|d:

    out[b] = sum_{q,d} TsM[q,d] * H[q,p_b] * H[d,c_b]
           = sum_d v[d,b] * M1[d,b]

where TsM = theta_spread.reshape(64,64), H = Sylvester-Hadamard-64,
A = (-2H) @ TsM (the only theta-side compute: ONE 64x64 matmul),
M1[d,b] = A[p_b,d] (a one-hot gather matmul over the p-side), and
v[d,b] = (-1)^{popcount(d & c_b)} enters as vhat = parity(pc) - 0.5 = -v/2
(the -2 is folded into H on the host).

parity(pc) is built EXACTLY with fp32 round-to-nearest-even (HW-verified;
`mod` is not a legal tensor_scalar op on TRN2):
  - a third key matmul computes q1 = pc/2 - 0.25 + 2^23 whose fp32 PSUM
    accumulation rounds to exactly 2^23 + floor(pc/2) (the two constants ride
    two bf16-exact ones-rows, the big one last in partition order);
  - s1 = -2*q1 + 2^24 = -2*floor(pc/2)            (Activation, Copy w/ scale)
  - vhat = (pc - 0.5) + s1 = parity - 0.5          (DVE scalar_tensor_tensor)

The 512-sample batch is split into two groups whose bit rows sit at matmul
partition bases 0/32; each key matmul writes its group's PSUM partition half
directly ([128,256] stacks; a 64-col stationary may target out-partition base
64, while ACCUMULATING across different stationary bases hangs the HW).  The
stacked layout makes is_equal, vhat, and the product ONE 128-partition DVE op
each (DVE cost scales with the free axis only).

Per core (512 batch rows):
  1. ONE input DMA [64, 576] bf16 (SP/HWDGE): row-stacked bit groups,
     the three 64-col key stationaries, -2*H64, TsM.
  2. PE: warm-ups seed the p-state ramp; A-matmul; 6 key matmuls -> p_k
     (p(b)-j offsets), p_q1 (rounded half-counts), p_vk (popcount counts).
  3. DVE: stages A into the two gather stationary blocks (separate tiles:
     same-tile WAW costs a blocking self-sem), is_equal -> OHp stack,
     vhat via scalar_tensor_tensor; Act: s1 (its only op, so the scheduler
     cannot reorder it behind copies).
  4. PE: two gather matmuls -> M1 stack (bases 0/64); DVE: prod = M1 * vhat
     (a tiny spacer op sits between vhat and prod: Tile adds a blocking
     self-sem when an op reads its IMMEDIATE DVE predecessor's output);
     PE: four 1-column ones-matmuls column-reduce prod into p_out[128, 4]
     (sample g*128+i lands on partition i, column g).
  5. DVE stages p_out -> SBUF; a PREPARED kv_writeback (descriptors generated
     on Pool during the input-DMA wait) is fired by trigger_dma: the
     post-compute tail is trigger + transfer + DMA-sem, skipping the 625ns
     HWDGE gen + 650ns DGE->DMA delay of a plain dma_start.

Host does only sharding, dtype/layout staging, and index bookkeeping
(mask XOR-fold + theta scatter).  All theta- and bit-dependent arithmetic
runs on device.
"""

import numpy as np

B, NUM_BITS, ORDER = 4096, 32, 12
N_CORES = 8
B_LOCAL = B // N_CORES          # 512
KEYS = 1 << ORDER               # 4096
PC = 64                         # 6/6 split: p = key>>6, c = key&63
PC_BITS = 6
GROUPS = 2                      # batch groups row-stacked at partition 0 / 32
GROW = 32                       # group g rows start at g*GROW (matmul base rule)
GB = B_LOCAL // GROUPS          # 256 samples per group
ROWS = PC_BITS * 2 + 2          # 12 bit rows + two ones rows = 14
IN_COLS = GB + 5 * PC           # bits 256 | statP 64 | statQ 64 | statC 64 | H | Ts

_STATE = {}


def _sylvester(n):
    """H[i,j] = (-1)^popcount(i&j), Sylvester ordering."""
    h = np.array([[1.0]], dtype=np.float32)
    while h.shape[0] < n:
        h = np.block([[h, h], [h, -h]])
    return np.ascontiguousarray(h, dtype=np.float32)


def _build_module():
    import bass_rust as _bass_rust
    import concourse.mybir as mybir
    import concourse.tile as tile
    from concourse import bacc

    f32 = mybir.dt.float32
    bf16 = mybir.dt.bfloat16
    i32 = mybir.dt.int32
    nc = bacc.Bacc(
        "TRN2",
        target_bir_lowering=False,
        debug=False,
        enable_asserts=True,
        num_devices=N_CORES,
    )

    inp = nc.dram_tensor("inp", [PC, IN_COLS], bf16, kind="ExternalInput").ap()
    # kv layout [batch=1, d_head=128, d_head_outer=1, n_ctx=4] is flat
    # f32[p*4+g] == p_out[p, g], identical to a plain [128, 4] tensor.
    out = nc.dram_tensor("out", [1, 128, 1, 4], f32, kind="ExternalOutput").ap()

    C_SP = GB                  # stat-P block (p-offsets)
    C_SQ = GB + PC             # stat-Q block: pc/2 - 0.25 + 2^23 (RNE floor)
    C_SC = GB + 2 * PC         # stat-C block (popcount weights)
    C_H = GB + 3 * PC          # -2*H64
    C_TS = GB + 4 * PC         # TsM

    with tile.TileContext(nc) as tc:
        with (
            tc.tile_pool(name="sb", bufs=1) as sb,
            tc.tile_pool(name="ps", bufs=1, space="PSUM") as ps,
        ):
            # --- pre-DMA setup: constants + PE p-state warm-up ---------------
            t_w = sb.tile([1, 1], bf16)
            nc.vector.memset(t_w, 1.0)
            t_ones = sb.tile([128, 1], f32)
            nc.vector.memset(t_ones, 1.0)
            t_A0 = sb.tile([PC, PC], bf16)
            t_A1 = sb.tile([128, PC], bf16)
            t_out = sb.tile([128, 1, 1, 4], f32)
            t_ctx = sb.tile([128, 1], i32)
            nc.gpsimd.memset(t_ctx, 0)
            dma_sem = nc.alloc_semaphore("out_dma")
            nc.gpsimd.kv_writeback(
                out, t_out[:], t_ctx[:], prepare_only=True, sem=dma_sem)
            # warm-ups target p_out (overwritten by the column sums later);
            # PSUM has only 8 banks and each tile takes a full bank
            p_out = ps.tile([128, 4], f32)
            nc.tensor.matmul(p_out[0:1, 0:1], t_w, t_w)
            nc.tensor.matmul(p_out[0:1, 0:1], t_w, t_w)

            t_in = sb.tile([PC, IN_COLS], bf16)
            nc.sync.dma_start(out=t_in, in_=inp)

            # --- theta side: A = (-2H) @ TsM, staged into the two diagonal
            # blocks of the M1 stationary (Act; GPSIMD can't touch PSUM)
            p_A = ps.tile([PC, PC], f32)
            nc.tensor.matmul(p_A, t_in[:, C_H : C_H + PC], t_in[:, C_TS : C_TS + PC])
            # both gather stationary blocks staged by DVE in its idle window
            # before is_equal's input is ready (the second p_A read carries a
            # ~160ns reader-chain guard on ANY engine; DVE's window hides it)
            nc.vector.tensor_copy(t_A0, p_A)
            nc.vector.tensor_copy(t_A1[PC : 2 * PC, :], p_A)

            # --- bits side: key matmuls stack group0 on PSUM partitions
            # 0:64 and group1 on 64:128 (64-col stationaries; the PSUM out AP
            # carries the partition base -- HW-verified, accumulation across
            # different stationary bases is NOT)
            p_k = ps.tile([2 * PC, GB], f32)
            p_q1 = ps.tile([2 * PC, GB], f32)
            p_vk = ps.tile([2 * PC, GB], f32)
            r0 = slice(0, ROWS)
            r1 = slice(GROW, GROW + ROWS)
            for p_dst, c_base in ((p_q1, C_SQ), (p_k, C_SP), (p_vk, C_SC)):
                nc.tensor.matmul(p_dst[0:PC, :], t_in[r0, c_base : c_base + PC],
                                 t_in[r0, 0:GB])
                nc.tensor.matmul(p_dst[PC : 2 * PC, :],
                                 t_in[r1, c_base : c_base + PC],
                                 t_in[r1, 0:GB])

            t_oh = sb.tile([2 * PC, GB], bf16)
            nc.vector.tensor_scalar(
                out=t_oh, in0=p_k,
                scalar1=0.0, scalar2=None, op0=mybir.AluOpType.is_equal)
            # sign contraction operand: vhat = parity(pc) - 0.5, parity from
            # the fp32 round-to-nearest-even floor baked into the q1 matmul:
            #   q1(PSUM) = 2^23 + floor(pc/2)   (exactly; HW-verified)
            #   s1 = q1*-2 + 2^24 = -2*floor(pc/2)
            #   vhat = (pc - 0.5) + s1 = parity - 0.5  (exact, in {-.5, +.5})
            t_s1 = sb.tile([2 * PC, GB], f32)
            nc.scalar.activation(
                t_s1, p_q1, mybir.ActivationFunctionType.Copy,
                bias=float(2.0**24), scale=-2.0)
            t_vst = sb.tile([2 * PC, GB], f32)
            nc.vector.scalar_tensor_tensor(
                out=t_vst, in0=p_vk, scalar=-0.5, in1=t_s1,
                op0=mybir.AluOpType.add, op1=mybir.AluOpType.add)

            # --- gather + sign-contraction: out[b] = sum_d M1[d,b]*v[d,b] ----
            p_M1 = ps.tile([2 * PC, GB], f32)
            nc.tensor.matmul(p_M1[PC : 2 * PC, :], t_A1[PC : 2 * PC, :],
                             t_oh[PC : 2 * PC, :])
            nc.tensor.matmul(p_M1[0:PC, :], t_A0, t_oh[0:PC, :])
            # tiny spacer op: Tile inserts a blocking self-sem when an op
            # reads its IMMEDIATE DVE predecessor's output; with the spacer
            # the vhat write is two ops back and prod dispatches freely
            t_dmy = sb.tile([1, 1], f32)
            nc.vector.tensor_copy(t_dmy, t_s1[0:1, 0:1])
            t_prod = sb.tile([2 * PC, GB], f32)
            nc.vector.tensor_mul(t_prod, p_M1, t_vst)

            # column sums, batch transposed onto output partitions: four
            # 1-column matmuls (stationary = a 128-column slice of prod)
            for g in range(4):
                rows = slice((g // 2) * PC, (g // 2 + 1) * PC)
                cols = slice((g % 2) * 128, (g % 2 + 1) * 128)
                nc.tensor.matmul(p_out[:, g : g + 1],
                                 t_prod[rows, cols], t_ones[rows, :])

            # the trigger is sequencer-only and Tile's deferred-RAW machinery
            # only covers producers issued BEFORE the prep, so attach an
            # explicit sync dependency on the staging copy
            cp = nc.vector.tensor_copy(t_out[:, 0, 0, :], p_out)
            trig = nc.gpsimd.trigger_dma(count=None)
            deps = _bass_rust.InstructionNameOrderedSet()
            deps.add(cp.ins.name)
            trig.ins.add_sync_dependencies_from(deps)
            wt = nc.gpsimd.wait_ge(dma_sem, 16)
            tdeps = _bass_rust.InstructionNameOrderedSet()
            tdeps.add(trig.ins.name)
            wt.ins.add_nosync_dependencies_from(tdeps)

    # Hoist the input DMA into the entry block, before SP's entry-barrier
    # wait: its HWDGE descriptor generation (625ns) + DGE->DMA delay (650ns)
    # touch no semaphores, nothing reads t_in before the barrier, and the
    # completion-sem update (~2.4us) lands long after Pool's sem-clear
    # memsets (~0.5us) -- so the whole DMA pipeline overlaps the ~640ns
    # entry barrier instead of serializing behind it.
    blocks = list(nc.m.functions[0].blocks)
    main_il = blocks[0].instructions
    tile_il = blocks[1].instructions
    dma_idx = next(i for i, inst in enumerate(tile_il)
                   if type(inst).__name__.endswith("InstDMACopy"))
    dma_inst = tile_il.pop(dma_idx)
    sp_bar = next(i for i, inst in enumerate(main_il)
                  if inst.name.startswith("barrier_SP_"))
    main_il.insert(sp_bar, dma_inst)

    # The exit block ends with TWO full all-engine barrier rounds; round 2
    # only fences the sem-range-clear against engine halt, but every engine's
    # stream ends right after round 1 (nothing races the clear, launches are
    # serialized by all-engines-halted, and round 1 leaves the barrier sems
    # at 0 -- the state the next launch's entry expects).  Truncate after the
    # clear (the last Pool ISA instruction).
    end_blk = list(nc.m.functions[0].blocks)[-1]
    il = end_blk.instructions
    isa_idx = max(i for i, inst in enumerate(il)
                  if type(inst).__name__.endswith("InstISA"))
    while len(il) > isa_idx + 1:
        il.pop()

    # Tile put the kv_writeback prep on the DMASW0 lane and emitted exit
    # waits on that lane's sem, but with a user completion sem (sem=) nothing
    # ever bumps it -> deadlock.  Pool's explicit wait_ge(out_dma)>=16 plus
    # the final all-engine barrier already guarantee the transfer landed
    # before teardown, so drop the orphaned lane waits.
    for blk in nc.m.functions[0].blocks:
        for inst in blk.instructions:
            si = inst.sync_info
            if si is None or not si.on_wait:
                continue
            if any(w.ant_name and w.ant_name.startswith("DMASW")
                   for w in si.on_wait):
                si.on_wait = [
                    w for w in si.on_wait
                    if not (w.ant_name and w.ant_name.startswith("DMASW"))
                ]

    nc.compile()
    return nc


def _get_module():
    nc = _STATE.get("nc")
    if nc is None:
        nc = _build_module()
        _STATE["nc"] = nc
    return nc


def _host_prep(bitstrings, theta, idx_pad):
    """Index bookkeeping + input staging. Returns per-core input maps."""
    import ml_dtypes

    bitstrings = np.asarray(bitstrings)
    theta = np.asarray(theta, dtype=np.float32)
    idx_pad = np.asarray(idx_pad).astype(np.int64)

    # mask[t] = XOR-fold of one-hot bit positions (pad index >= NUM_BITS -> no bit)
    onehots = np.where(idx_pad >= NUM_BITS, 0, np.int64(1) << np.clip(idx_pad, 0, 62))
    masks = np.bitwise_xor.reduce(onehots, axis=1)
    if masks.size and int(masks.max()) >= KEYS:
        raise NotImplementedError(
            "kernel specialized for masks spanning bits 0..11 "
            f"(max mask {int(masks.max())})"
        )
    theta_spread = np.zeros(KEYS, np.float32)
    np.add.at(theta_spread, masks, theta)
    ts_m = theta_spread.reshape(PC, PC)                 # TsM[q, d]

    # Key-matmul stationaries [14, 64] each (rows 12/13 are ones-rows):
    #   p-off[:, j] : p(b)-j  = sum_{k=6..11} 2^(k-6) bit_k  +  (-j)*1
    #   cnt[:, d]   : pc = popcount(d & c_b) = sum_{k=0..5} dbit_k bit_k
    #   q1[:, d]    : pc/2 - 0.25 + 2^23 -> fp32 PSUM rounds (RNE) to
    #                 2^23 + floor(pc/2); constants split across the two
    #                 ones-rows (each bf16-exact), 2^23 last in partition
    #                 order so the single rounding happens at the end
    w_p = np.zeros((ROWS, PC), np.float32)
    for k in range(PC_BITS):
        w_p[PC_BITS + k, :] = float(1 << k)
    w_p[2 * PC_BITS, :] = -np.arange(PC, dtype=np.float32)
    d_idx = np.arange(PC)
    w_c = np.zeros((ROWS, PC), np.float32)
    for k in range(PC_BITS):
        w_c[k, :] = ((d_idx >> k) & 1).astype(np.float32)
    w_q = 0.5 * w_c
    w_q[2 * PC_BITS, :] = -0.25
    w_q[2 * PC_BITS + 1, :] = float(2.0**23)

    h64 = _sylvester(PC)

    C_SP = GB
    C_SQ = GB + PC
    C_SC = GB + 2 * PC
    C_H = GB + 3 * PC
    C_TS = GB + 4 * PC

    base = np.zeros((PC, IN_COLS), np.float32)
    base[:, C_H : C_H + PC] = -2.0 * h64
    base[:, C_TS : C_TS + PC] = ts_m
    # group g's stationaries live at rows g*GROW (sharing the moving base)
    for g in range(GROUPS):
        rows = slice(g * GROW, g * GROW + ROWS)
        base[rows, C_SP : C_SP + PC] = w_p
        base[rows, C_SQ : C_SQ + PC] = w_q
        base[rows, C_SC : C_SC + PC] = w_c

    bits_f = bitstrings[:, :ORDER].astype(np.float32)
    in_maps = []
    for c in range(N_CORES):
        buf = base.copy()
        for g in range(GROUPS):
            rows = slice(g * GROW, g * GROW + ORDER)
            s0 = c * B_LOCAL + g * GB
            buf[rows, 0:GB] = bits_f[s0 : s0 + GB, :].T
            buf[g * GROW + ORDER, 0:GB] = 1.0
            buf[g * GROW + ORDER + 1, 0:GB] = 1.0
        in_maps.append({"inp": buf.astype(ml_dtypes.bfloat16)})
    return in_maps


def kernel(bitstrings, theta, idx_pad):
    from concourse.bass_utils import run_bass_kernel_spmd

    in_maps = _host_prep(bitstrings, theta, idx_pad)
    nc = _get_module()
    res = run_bass_kernel_spmd(nc, in_maps, core_ids=list(range(N_CORES)))
    # out flat f32[i*4 + g] holds sample b_local = g*128 + i
    out = np.concatenate(
        [np.asarray(r["out"]).reshape(128, 4).T.ravel() for r in res.results])
    return out.astype(np.float32)
